# revision 3
# baseline (speedup 1.0000x reference)
"""Local (sliding-window) attention on 8 Trainium2 NeuronCores.

Problem: B=2, T=2048, H=8, E=64, local_context C=128.
Query i attends keys [i-64, i+64) (clipped to [0, T)).

Sharding: the 16 (b,h) pairs are split 2-per-core (pure data parallel,
no halo needed).

Per-core kernel (2 head-pairs, T=2048):
  - Host pre-transposes Q,K to [E, T] and packs the two pairs onto the
    128 SBUF partitions (pair0 -> partitions 0:64, pair1 -> 64:128), so
    no on-chip transposes are needed and DMAs are full-width.
  - Key chunks C_j = keys [128j+64, 128j+192), j = -1..15 (edges clamped
    to [0,128) / [1920,2048)). Each query tile i in [0,16) covers
    queries [128i, 128i+128) and needs chunks C_{i-1} (A) and C_i (B).
  - Scores computed transposed: S^T[key, query] = matmul(lhsT=K^T chunk,
    rhs=Q^T two query tiles) -> one [128, 256] PSUM tile per chunk; the
    stationary K^T chunk load is shared by both query tiles.
  - Band masks (additive -1e9) are added in PSUM by DVE, then ACT does
    exp(0.125 * x) PSUM->SBUF.
  - AV: out[query, e] accumulates matmul(lhsT=expS^T part, rhs=V chunk)
    over the A and B parts. expS^T is already the correct stationary
    orientation, so no P transpose. V chunks carry a 65th column of
    ones, so column 64 of the PSUM accumulator is the softmax
    denominator for free.
  - Normalize: DVE reciprocal of col 64, tensor_scalar_mul, DMA out.
"""

import numpy as np

B, T, H, E = 2, 2048, 8, 64
C = 128
HALF = C // 2  # 64
NEG = -1e9
NT = T // 128  # 16 query tiles per head
NCORES = 8
PAIRS_PER_CORE = (B * H) // NCORES  # 2

_cache = {}


def _build():
    import concourse.bass as bass
    import concourse.mybir as mybir
    import concourse.tile as tile
    from concourse import bacc

    f32 = mybir.dt.float32
    AF = mybir.ActivationFunctionType
    ALU = mybir.AluOpType

    nc = bacc.Bacc("TRN2", target_bir_lowering=False, debug=False)
    qt_d = nc.dram_tensor("qt", (128, T), f32, kind="ExternalInput").ap()
    kt_d = nc.dram_tensor("kt", (128, T), f32, kind="ExternalInput").ap()
    v_d = nc.dram_tensor("v", (PAIRS_PER_CORE, T, E), f32, kind="ExternalInput").ap()
    o_d = nc.dram_tensor("o", (PAIRS_PER_CORE, T, E), f32, kind="ExternalOutput").ap()

    with tile.TileContext(nc) as tc:
        with (
            tc.tile_pool(name="const", bufs=1) as cpool,
            tc.tile_pool(name="io", bufs=1) as iopool,
            tc.tile_pool(name="es", bufs=4) as espool,
            tc.tile_pool(name="small", bufs=4) as spool,
            tc.tile_pool(name="ps_s", bufs=3, space="PSUM") as ps_s,
            tc.tile_pool(name="ps_o", bufs=4, space="PSUM") as ps_o,
        ):
            # ---- band masks (built once on POOL engine) ----
            # interior chunk mask over [128 keys x 256 queries]:
            # key row p <-> global key 128j+64+p; col c <-> query 128j+c.
            # valid iff c-128 <= p <= c-1.
            m_int = cpool.tile([128, 256], f32, tag="m_int")
            nc.gpsimd.memset(m_int[:], 0.0)
            nc.gpsimd.affine_select(
                out=m_int[:], in_=m_int[:], compare_op=ALU.is_ge, fill=NEG,
                base=-1, channel_multiplier=-1, pattern=[[1, 256]],
            )  # keep where c - p - 1 >= 0
            nc.gpsimd.affine_select(
                out=m_int[:], in_=m_int[:], compare_op=ALU.is_ge, fill=NEG,
                base=128, channel_multiplier=1, pattern=[[-1, 256]],
            )  # keep where p - c + 128 >= 0

            # first-tile A-part mask (chunk C_{-1} clamped to keys [0,128)):
            # row p <-> key p, col c <-> query c.
            # valid iff p <= 63 (avoid overlap with B-part) and p >= c-64.
            m_first = cpool.tile([128, 128], f32, tag="m_first")
            nc.gpsimd.memset(m_first[:], 0.0)
            nc.gpsimd.affine_select(
                out=m_first[:], in_=m_first[:], compare_op=ALU.is_ge, fill=NEG,
                base=63, channel_multiplier=-1, pattern=[[0, 128]],
            )  # keep where 63 - p >= 0
            nc.gpsimd.affine_select(
                out=m_first[:], in_=m_first[:], compare_op=ALU.is_ge, fill=NEG,
                base=64, channel_multiplier=1, pattern=[[-1, 128]],
            )  # keep where p - c + 64 >= 0

            # last-tile B-part mask (chunk C_15 clamped to keys [1920,2048)):
            # row p <-> key 1920+p, col c <-> query 1920+c.
            # valid iff p >= 64 (avoid overlap with A-part) and p <= c+63.
            m_last = cpool.tile([128, 128], f32, tag="m_last")
            nc.gpsimd.memset(m_last[:], 0.0)
            nc.gpsimd.affine_select(
                out=m_last[:], in_=m_last[:], compare_op=ALU.is_ge, fill=NEG,
                base=-64, channel_multiplier=1, pattern=[[0, 128]],
            )  # keep where p - 64 >= 0
            nc.gpsimd.affine_select(
                out=m_last[:], in_=m_last[:], compare_op=ALU.is_ge, fill=NEG,
                base=63, channel_multiplier=-1, pattern=[[1, 128]],
            )  # keep where c - p + 63 >= 0

            # ---- resident data tiles ----
            qt_sb = iopool.tile([128, T], f32, tag="qt")
            kt_sb = iopool.tile([128, T], f32, tag="kt")
            v_sb = [iopool.tile([128, NT + 1, E + 1], f32, tag=f"v{p}", name=f"v_sb{p}")
                    for p in range(PAIRS_PER_CORE)]
            o_sb = [iopool.tile([128, NT, E], f32, tag=f"o{p}", name=f"o_sb{p}")
                    for p in range(PAIRS_PER_CORE)]

            # ones column for the softmax denominator
            for p in range(PAIRS_PER_CORE):
                nc.gpsimd.memset(v_sb[p][:, :, E:E + 1], 1.0)

            # v DRAM view for interior slots: slot k (1..15) holds keys
            # [128k-64, 128k+64) -> rows 64.. of v, reshaped (k p) e.
            v_mid = [
                v_d[p, HALF:HALF + (NT - 1) * 128, :].rearrange(
                    "(k p) e -> p k e", p=128)
                for p in range(PAIRS_PER_CORE)
            ]
            o_r = [o_d[p].rearrange("(i p) e -> p i e", p=128)
                   for p in range(PAIRS_PER_CORE)]

            QW = 512  # input streaming quarter width (columns of qt/kt)

            es_tiles = {}

            def load_quarter(qq):
                c0 = qq * QW
                nc.sync.dma_start(kt_sb[:, c0:c0 + QW], kt_d[:, c0:c0 + QW])
                nc.sync.dma_start(qt_sb[:, c0:c0 + QW], qt_d[:, c0:c0 + QW])
                for p in range(PAIRS_PER_CORE):
                    if qq == 0:
                        # edge slot 0: keys [0, 128)
                        nc.sync.dma_start(v_sb[p][:, 0, :E], v_d[p, 0:128, :])
                        nc.sync.dma_start(v_sb[p][:, 1:5, :E], v_mid[p][:, 0:4, :])
                    elif qq == 3:
                        nc.sync.dma_start(v_sb[p][:, 13:16, :E], v_mid[p][:, 12:15, :])
                        # edge slot 16: keys [1920, 2048)
                        nc.sync.dma_start(v_sb[p][:, NT, :E], v_d[p, T - 128:T, :])
                    else:
                        s = 4 * qq + 1
                        nc.sync.dma_start(v_sb[p][:, s:s + 4, :E],
                                          v_mid[p][:, s - 1:s + 3, :])

            def scores(p, j):
                """chunk C_j for head-pair p -> exp(masked scores) in SBUF."""
                pb = 64 * p
                if j == -1:
                    k0, q0, nq, oc = 0, 0, 128, 128
                elif j == NT - 1:
                    k0, q0, nq, oc = T - 128, T - 128, 128, 0
                else:
                    k0, q0, nq, oc = 128 * j + HALF, 128 * j, 256, 0
                ps = ps_s.tile([128, 256], f32, tag="ps_s", name="ps")
                nc.tensor.matmul(
                    ps[:, oc:oc + nq],
                    kt_sb[pb:pb + 64, k0:k0 + 128],
                    qt_sb[pb:pb + 64, q0:q0 + nq],
                    start=True, stop=True,
                )
                if j == -1:
                    nc.vector.tensor_add(ps[:, 128:256], ps[:, 128:256], m_first[:])
                elif j == NT - 1:
                    nc.vector.tensor_add(ps[:, 0:128], ps[:, 0:128], m_last[:])
                else:
                    nc.vector.tensor_add(ps[:, :], ps[:, :], m_int[:])
                es = espool.tile([128, 256], f32, tag="es", name="es")
                nc.scalar.activation(es[:, oc:oc + nq], ps[:, oc:oc + nq],
                                     AF.Exp, scale=1.0 / np.sqrt(E))
                es_tiles[(p, j)] = es

            def av(p, i):
                """finalize query tile i of head-pair p."""
                po = ps_o.tile([128, E + 1], f32, tag="ps_o", name="po")
                es_a = es_tiles[(p, i - 1)]
                es_b = es_tiles[(p, i)]
                nc.tensor.matmul(po[:], es_a[:, 128:256], v_sb[p][:, i, :],
                                 start=True, stop=False)
                nc.tensor.matmul(po[:], es_b[:, 0:128], v_sb[p][:, i + 1, :],
                                 start=False, stop=True)
                rec = spool.tile([128, 1], f32, tag="rec", name="rec")
                nc.vector.reciprocal(rec[:], po[:, E:E + 1])
                nc.vector.tensor_scalar_mul(o_sb[p][:, i, :], po[:, 0:E], rec[:])

            # ---- software-pipelined schedule ----
            # quarter qq makes chunks j <= 4*qq + 2 runnable (kt needs cols
            # 128j+192 <= 512*(qq+1); qt needs 128j+256 <= 512*(qq+1)).
            load_quarter(0)
            for qq in range(1, 4):
                load_quarter(qq)
                lo = -1 if qq == 1 else 4 * (qq - 1) - 1
                hi = 4 * qq - 1
                for j in range(lo, hi):
                    for p in range(PAIRS_PER_CORE):
                        scores(p, j)
                        if j >= 1:
                            av(p, j - 1)
            for j in range(11, NT):
                for p in range(PAIRS_PER_CORE):
                    scores(p, j)
                    av(p, j - 1)
            for p in range(PAIRS_PER_CORE):
                av(p, NT - 1)
                nc.sync.dma_start(o_r[p][:, :, :], o_sb[p][:, :, :])

    nc.compile()
    return nc


def _get_nc():
    if "nc" not in _cache:
        _cache["nc"] = _build()
    return _cache["nc"]


def kernel(query, key, value, local_context):
    from concourse import bass_utils

    assert int(local_context) == C
    assert query.shape == (B, T, H, E)
    nc = _get_nc()

    # (B,T,H,E) -> (B*H, T, E)
    qh = np.ascontiguousarray(query.transpose(0, 2, 1, 3)).reshape(B * H, T, E)
    kh = np.ascontiguousarray(key.transpose(0, 2, 1, 3)).reshape(B * H, T, E)
    vh = np.ascontiguousarray(value.transpose(0, 2, 1, 3)).reshape(B * H, T, E)

    in_maps = []
    for c in range(NCORES):
        p0 = PAIRS_PER_CORE * c
        qt = np.ascontiguousarray(
            qh[p0:p0 + PAIRS_PER_CORE].transpose(0, 2, 1).reshape(128, T))
        kt = np.ascontiguousarray(
            kh[p0:p0 + PAIRS_PER_CORE].transpose(0, 2, 1).reshape(128, T))
        v = np.ascontiguousarray(vh[p0:p0 + PAIRS_PER_CORE])
        in_maps.append({"qt": qt, "kt": kt, "v": v})

    res = bass_utils.run_bass_kernel_spmd(nc, in_maps, core_ids=list(range(NCORES)))
    _cache["last_results"] = res

    oh = np.empty((B * H, T, E), dtype=np.float32)
    for c in range(NCORES):
        p0 = PAIRS_PER_CORE * c
        oh[p0:p0 + PAIRS_PER_CORE] = res.results[c]["o"]
    out = oh.reshape(B, H, T, E).transpose(0, 2, 1, 3)
    return np.ascontiguousarray(out)


# revision 5
# speedup vs baseline: 2.3555x; 2.3555x over previous
"""Local (sliding-window) attention on 8 Trainium2 NeuronCores.

Problem: B=2, T=2048, H=8, E=64, local_context C=128.
Query i attends keys [i-64, i+64) (clipped to [0, T)).

Sharding: the 16 (b,h) pairs are split 2-per-core (pure data parallel,
no halo needed).

Per-core kernel (2 head-pairs, T=2048):
  - Host pre-transposes Q,K to [E, T] and packs the two pairs onto the
    128 SBUF partitions (pair0 -> partitions 0:64, pair1 -> 64:128), so
    no on-chip transposes are needed and DMAs are full-width.
  - Key chunks C_j = keys [128j+64, 128j+192), j = -1..15 (edges clamped
    to [0,128) / [1920,2048)). Each query tile i in [0,16) covers
    queries [128i, 128i+128) and needs chunks C_{i-1} (A) and C_i (B).
  - Scores computed transposed: S^T[key, query] = matmul(lhsT=K^T chunk,
    rhs=Q^T two query tiles) -> one [128, 256] PSUM tile per chunk; the
    stationary K^T chunk load is shared by both query tiles.
  - Band masks (additive -1e9) are added in PSUM by DVE, then ACT does
    exp(0.125 * x) PSUM->SBUF.
  - AV: out[query, e] accumulates matmul(lhsT=expS^T part, rhs=V chunk)
    over the A and B parts. expS^T is already the correct stationary
    orientation, so no P transpose. V chunks carry a 65th column of
    ones, so column 64 of the PSUM accumulator is the softmax
    denominator for free.
  - Normalize: DVE reciprocal of col 64, tensor_scalar_mul, DMA out.
"""

import numpy as np

B, T, H, E = 2, 2048, 8, 64
C = 128
HALF = C // 2  # 64
NEG = -1e9
NT = T // 128  # 16 query tiles per head
NCORES = 8
PAIRS_PER_CORE = (B * H) // NCORES  # 2

_cache = {}


def _build():
    import concourse.bass as bass
    import concourse.mybir as mybir
    import concourse.tile as tile
    from concourse import bacc

    f32 = mybir.dt.float32
    bf16 = mybir.dt.bfloat16
    AF = mybir.ActivationFunctionType
    ALU = mybir.AluOpType

    nc = bacc.Bacc("TRN2", target_bir_lowering=False, debug=False)
    qt_d = nc.dram_tensor("qt", (128, T), bf16, kind="ExternalInput").ap()
    kt_d = nc.dram_tensor("kt", (128, T), bf16, kind="ExternalInput").ap()
    v_d = nc.dram_tensor("v", (PAIRS_PER_CORE, T, E), bf16, kind="ExternalInput").ap()
    o_d = nc.dram_tensor("o", (PAIRS_PER_CORE, T, E), f32, kind="ExternalOutput").ap()

    with tile.TileContext(nc) as tc:
        with (
            tc.tile_pool(name="const", bufs=1) as cpool,
            tc.tile_pool(name="io", bufs=1) as iopool,
            tc.tile_pool(name="es", bufs=4) as espool,
            tc.tile_pool(name="small", bufs=4) as spool,
            tc.tile_pool(name="ps_s", bufs=3, space="PSUM") as ps_s,
            tc.tile_pool(name="ps_o", bufs=4, space="PSUM") as ps_o,
        ):
            # ---- band masks (built once on POOL engine) ----
            # interior chunk mask over [128 keys x 256 queries]:
            # key row p <-> global key 128j+64+p; col c <-> query 128j+c.
            # valid iff c-128 <= p <= c-1.
            m_int = cpool.tile([128, 256], bf16, tag="m_int")
            nc.gpsimd.memset(m_int[:], 1.0)
            nc.gpsimd.affine_select(
                out=m_int[:], in_=m_int[:], compare_op=ALU.is_ge, fill=0.0,
                base=-1, channel_multiplier=-1, pattern=[[1, 256]],
            )  # keep where c - p - 1 >= 0
            nc.gpsimd.affine_select(
                out=m_int[:], in_=m_int[:], compare_op=ALU.is_ge, fill=0.0,
                base=128, channel_multiplier=1, pattern=[[-1, 256]],
            )  # keep where p - c + 128 >= 0

            # first-tile A-part mask (chunk C_{-1} clamped to keys [0,128)):
            # row p <-> key p, col c <-> query c.
            # valid iff p <= 63 (avoid overlap with B-part) and p >= c-64.
            m_first = cpool.tile([128, 128], bf16, tag="m_first")
            nc.gpsimd.memset(m_first[:], 1.0)
            nc.gpsimd.affine_select(
                out=m_first[:], in_=m_first[:], compare_op=ALU.is_ge, fill=0.0,
                base=63, channel_multiplier=-1, pattern=[[0, 128]],
            )  # keep where 63 - p >= 0
            nc.gpsimd.affine_select(
                out=m_first[:], in_=m_first[:], compare_op=ALU.is_ge, fill=0.0,
                base=64, channel_multiplier=1, pattern=[[-1, 128]],
            )  # keep where p - c + 64 >= 0

            # last-tile B-part mask (chunk C_15 clamped to keys [1920,2048)):
            # row p <-> key 1920+p, col c <-> query 1920+c.
            # valid iff p >= 64 (avoid overlap with A-part) and p <= c+63.
            m_last = cpool.tile([128, 128], bf16, tag="m_last")
            nc.gpsimd.memset(m_last[:], 1.0)
            nc.gpsimd.affine_select(
                out=m_last[:], in_=m_last[:], compare_op=ALU.is_ge, fill=0.0,
                base=-64, channel_multiplier=1, pattern=[[0, 128]],
            )  # keep where p - 64 >= 0
            nc.gpsimd.affine_select(
                out=m_last[:], in_=m_last[:], compare_op=ALU.is_ge, fill=0.0,
                base=63, channel_multiplier=-1, pattern=[[1, 128]],
            )  # keep where c - p + 63 >= 0

            # ---- resident data tiles ----
            qt_sb = iopool.tile([128, T], bf16, tag="qt")
            kt_sb = iopool.tile([128, T], bf16, tag="kt")
            v_sb = [iopool.tile([128, NT + 1, E + 1], bf16, tag=f"v{p}", name=f"v_sb{p}")
                    for p in range(PAIRS_PER_CORE)]
            o_sb = [iopool.tile([128, NT, E], f32, tag=f"o{p}", name=f"o_sb{p}")
                    for p in range(PAIRS_PER_CORE)]

            # ones column for the softmax denominator
            for p in range(PAIRS_PER_CORE):
                nc.gpsimd.memset(v_sb[p][:, :, E:E + 1], 1.0)

            # v DRAM view for interior slots: slot k (1..15) holds keys
            # [128k-64, 128k+64) -> rows 64.. of v, reshaped (k p) e.
            v_mid = [
                v_d[p, HALF:HALF + (NT - 1) * 128, :].rearrange(
                    "(k p) e -> p k e", p=128)
                for p in range(PAIRS_PER_CORE)
            ]
            o_r = [o_d[p].rearrange("(i p) e -> p i e", p=128)
                   for p in range(PAIRS_PER_CORE)]

            QW = 512  # input streaming quarter width (columns of qt/kt)

            es_tiles = {}

            def load_quarter(qq):
                c0 = qq * QW
                nc.sync.dma_start(kt_sb[:, c0:c0 + QW], kt_d[:, c0:c0 + QW])
                nc.sync.dma_start(qt_sb[:, c0:c0 + QW], qt_d[:, c0:c0 + QW])
                for p in range(PAIRS_PER_CORE):
                    if qq == 0:
                        # edge slot 0: keys [0, 128)
                        nc.sync.dma_start(v_sb[p][:, 0, :E], v_d[p, 0:128, :])
                        nc.sync.dma_start(v_sb[p][:, 1:5, :E], v_mid[p][:, 0:4, :])
                    elif qq == 3:
                        nc.sync.dma_start(v_sb[p][:, 13:16, :E], v_mid[p][:, 12:15, :])
                        # edge slot 16: keys [1920, 2048)
                        nc.sync.dma_start(v_sb[p][:, NT, :E], v_d[p, T - 128:T, :])
                    else:
                        s = 4 * qq + 1
                        nc.sync.dma_start(v_sb[p][:, s:s + 4, :E],
                                          v_mid[p][:, s - 1:s + 3, :])

            def scores(p, j):
                """chunk C_j for head-pair p -> exp(masked scores) in SBUF."""
                pb = 64 * p
                if j == -1:
                    k0, q0, nq, oc = 0, 0, 128, 128
                elif j == NT - 1:
                    k0, q0, nq, oc = T - 128, T - 128, 128, 0
                else:
                    k0, q0, nq, oc = 128 * j + HALF, 128 * j, 256, 0
                ps = ps_s.tile([128, 256], f32, tag="ps_s", name="ps")
                nc.tensor.matmul(
                    ps[:, oc:oc + nq],
                    kt_sb[pb:pb + 64, k0:k0 + 128],
                    qt_sb[pb:pb + 64, q0:q0 + nq],
                    start=True, stop=True,
                )
                es = espool.tile([128, 256], bf16, tag="es", name="es")
                nc.scalar.activation(es[:, oc:oc + nq], ps[:, oc:oc + nq],
                                     AF.Exp, scale=1.0 / np.sqrt(E))
                if j == -1:
                    nc.vector.tensor_mul(es[:, 128:256], es[:, 128:256], m_first[:])
                elif j == NT - 1:
                    nc.vector.tensor_mul(es[:, 0:128], es[:, 0:128], m_last[:])
                else:
                    nc.vector.tensor_mul(es[:, :], es[:, :], m_int[:])
                es_tiles[(p, j)] = es

            def av(p, i):
                """finalize query tile i of head-pair p."""
                po = ps_o.tile([128, E + 1], f32, tag="ps_o", name="po")
                es_a = es_tiles[(p, i - 1)]
                es_b = es_tiles[(p, i)]
                nc.tensor.matmul(po[:], es_a[:, 128:256], v_sb[p][:, i, :],
                                 start=True, stop=False)
                nc.tensor.matmul(po[:], es_b[:, 0:128], v_sb[p][:, i + 1, :],
                                 start=False, stop=True)
                rec = spool.tile([128, 1], f32, tag="rec", name="rec")
                nc.vector.reciprocal(rec[:], po[:, E:E + 1])
                nc.vector.tensor_scalar_mul(o_sb[p][:, i, :], po[:, 0:E], rec[:])

            # ---- software-pipelined schedule ----
            # quarter qq makes chunks j <= 4*qq + 2 runnable (kt needs cols
            # 128j+192 <= 512*(qq+1); qt needs 128j+256 <= 512*(qq+1)).
            load_quarter(0)
            for qq in range(1, 4):
                load_quarter(qq)
                lo = -1 if qq == 1 else 4 * (qq - 1) - 1
                hi = 4 * qq - 1
                for j in range(lo, hi):
                    for p in range(PAIRS_PER_CORE):
                        scores(p, j)
                        if j >= 1:
                            av(p, j - 1)
            for j in range(11, NT):
                for p in range(PAIRS_PER_CORE):
                    scores(p, j)
                    av(p, j - 1)
            for p in range(PAIRS_PER_CORE):
                av(p, NT - 1)
                nc.sync.dma_start(o_r[p][:, :, :], o_sb[p][:, :, :])

    nc.compile()
    return nc


def _get_nc():
    if "nc" not in _cache:
        _cache["nc"] = _build()
    return _cache["nc"]


def kernel(query, key, value, local_context):
    import ml_dtypes
    from concourse import bass_utils

    assert int(local_context) == C
    assert query.shape == (B, T, H, E)
    nc = _get_nc()

    bf = ml_dtypes.bfloat16
    # (B,T,H,E) -> (B*H, T, E)
    qh = np.ascontiguousarray(query.transpose(0, 2, 1, 3)).reshape(B * H, T, E)
    kh = np.ascontiguousarray(key.transpose(0, 2, 1, 3)).reshape(B * H, T, E)
    vh = np.ascontiguousarray(value.transpose(0, 2, 1, 3)).reshape(B * H, T, E)

    in_maps = []
    for c in range(NCORES):
        p0 = PAIRS_PER_CORE * c
        qt = np.ascontiguousarray(
            qh[p0:p0 + PAIRS_PER_CORE].transpose(0, 2, 1).reshape(128, T)
        ).astype(bf)
        kt = np.ascontiguousarray(
            kh[p0:p0 + PAIRS_PER_CORE].transpose(0, 2, 1).reshape(128, T)
        ).astype(bf)
        v = np.ascontiguousarray(vh[p0:p0 + PAIRS_PER_CORE]).astype(bf)
        in_maps.append({"qt": qt, "kt": kt, "v": v})

    res = bass_utils.run_bass_kernel_spmd(nc, in_maps, core_ids=list(range(NCORES)))
    _cache["last_results"] = res

    oh = np.empty((B * H, T, E), dtype=np.float32)
    for c in range(NCORES):
        p0 = PAIRS_PER_CORE * c
        oh[p0:p0 + PAIRS_PER_CORE] = res.results[c]["o"]
    out = oh.reshape(B, H, T, E).transpose(0, 2, 1, 3)
    return np.ascontiguousarray(out)
